# revision 35
# baseline (speedup 1.0000x reference)
"""Block-sparse (view-causal) multi-head attention on 8 TRN2 NeuronCores.

Full inputs in, full output out. Sharding: data-parallel over batch (B=2),
tensor-parallel over heads (16 heads -> 4 per core). Each core computes its
4 heads' attention + its slice of the output projection; the host sums the
4 head-group partial projections per batch (the tensor-parallel reduce).

v2 layout: scores are computed transposed (key on partitions, query free),
exp'd on the scalar engine into SBUF, then the PV matmul is FLIPPED: the
stationary operand is V augmented with a ones column ([128 keys, 64+1]), the
exp tile is streamed at N=512. This (a) produces the attention output
directly transposed ([dh, q]) as the out-projection wants it -- no PE
transposes, (b) keeps the softmax denominator in psum row 64, and (c) turns
PV from LDWEIGHTS-bound (559 stationary loads of exp tiles) into
stream-bound. The denominator row is reciprocal'd (DVE), broadcast across
64 partitions (GpSimd, otherwise idle), and multiplied in during psum
evacuation. Adjacent causal views share key blocks, so their score/PV
streams are merged to 512-wide where possible. Exp runs on the scalar
engine from ~10us in (first score tile) to the end, overlapped with all
projection matmuls via manual emission interleaving (engines execute their
streams in order).
"""

import sys

if "/opt/trn_rl_repo" not in sys.path:
    sys.path.insert(0, "/opt/trn_rl_repo")

import numpy as np
import ml_dtypes

B, V, L, C, H = 2, 8, 256, 1024, 16
S = V * L                # 2048 tokens
DH = C // H              # 64
HPC = 4                  # heads per core
CPB = HPC * DH           # 256 channel block per core
N_CORES = 8
SCALE = DH ** -0.5       # 1/8, folded into the exp activation

_compiled = {}
LAST_RESULTS = None
EMIT_LOG = []            # (instruction-id watermark, label) for profiling
PACK_QK = True           # unused (kept for test.py compat)
SAFE_RECIP = False
SPLIT_ACT = False
N_WARMUP = 24            # junk matmuls to cover input DMA + HAM warmup


def _group_steps(g):
    """Score/PV steps for query group g (views 2g, 2g+1).

    Returns (qbase, steps) where each step is (kv, pvoff, w):
    kv = key view streamed, pvoff = column offset in the [65, 512] PV psum,
    w = query width. qbase is the global query offset of the group.
    """
    if g == 0:
        # view 0 attends view 1 only; view 1 attends view 0 only
        return 0, [(1, 0, 256), (0, 256, 256)]
    qa = 2 * g
    steps = [(kv, 0, 512) for kv in range(qa + 1)]   # shared causal blocks
    steps.append((qa + 1, 256, 256))                 # extra diag for view qa+1
    return qa * 256, steps


def build():
    import concourse.tile as tile
    from concourse import bacc, mybir, library_config

    f32 = mybir.dt.float32
    bf16 = mybir.dt.bfloat16
    EXP = mybir.ActivationFunctionType.Exp
    MULT = mybir.AluOpType.mult

    nc = bacc.Bacc("TRN2", target_bir_lowering=False, debug=False,
                   num_devices=N_CORES)
    xT = nc.dram_tensor("xT", [C, S], bf16, kind="ExternalInput").ap()
    wqT = nc.dram_tensor("wqT", [C, CPB], bf16, kind="ExternalInput").ap()
    wkT = nc.dram_tensor("wkT", [C, CPB], bf16, kind="ExternalInput").ap()
    wvT = nc.dram_tensor("wvT", [C, CPB], bf16, kind="ExternalInput").ap()
    wpT = nc.dram_tensor("wpT", [CPB, C], bf16, kind="ExternalInput").ap()
    y = nc.dram_tensor("y", [S, C], bf16, kind="ExternalOutput").ap()

    KC = C // 128        # 8 contraction chunks for the projections
    NS = S // 512        # 4 free-dim chunks for q/k projections
    SC = S // 128        # 16 sequence chunks

    with tile.TileContext(nc) as tc:
        with (
            tc.tile_pool(name="xt", bufs=1) as xt_pool,
            tc.tile_pool(name="wts", bufs=1) as w_pool,
            tc.tile_pool(name="qk", bufs=1) as qk_pool,
            tc.tile_pool(name="va", bufs=SC) as va_pool,
            tc.tile_pool(name="ot", bufs=1) as ot_pool,
            tc.tile_pool(name="exp", bufs=8) as exp_pool,
            tc.tile_pool(name="nrm", bufs=1) as nrm_pool,
            tc.tile_pool(name="small", bufs=1) as small_pool,
            tc.tile_pool(name="ysb", bufs=3) as ysb_pool,
            tc.tile_pool(name="pse", bufs=2, space="PSUM") as psum_e,
            tc.tile_pool(name="ppj", bufs=2, space="PSUM") as psum_j,
            tc.tile_pool(name="ppv", bufs=2, space="PSUM") as psum_v,
        ):
            # ---- input DMAs. A dma_start trigger costs >1us on the issuing
            # engine's sequencer, so batch everything into a handful of
            # multi-tile transfers. Weights ride gpsimd (one DMA per
            # matrix); xT rides sync in three token-range pieces so the
            # first 512 tokens (q/k chunk 0 -> first scores -> first exp)
            # land as early as possible. The scalar engine issues NO input
            # DMAs -- it must start exp'ing immediately.
            xts_all = xt_pool.tile([128, KC * S], bf16, tag="xt",
                                   name="xts_all")
            xts = [xts_all[:, k * S:(k + 1) * S] for k in range(KC)]
            wq_all = w_pool.tile([128, KC * CPB], bf16, tag="wq", name="wqa")
            wk_all = w_pool.tile([128, KC * CPB], bf16, tag="wk", name="wka")
            wv_all = w_pool.tile([128, KC * CPB], bf16, tag="wv", name="wva")
            wp_all = w_pool.tile([128, 2 * C], bf16, tag="wp", name="wpa")
            wq_t = [wq_all[:, k * CPB:(k + 1) * CPB] for k in range(KC)]
            wk_t = [wk_all[:, k * CPB:(k + 1) * CPB] for k in range(KC)]
            wv_t = [wv_all[:, k * CPB:(k + 1) * CPB] for k in range(KC)]
            wp_t = [wp_all[:, k * C:(k + 1) * C] for k in range(2)]

            def dma_w(q, wall, wdram, nt, w):
                # partition dim FIRST on the SBUF side (dep tracking
                # interprets dim0 as partitions)
                q.dma_start(
                    wall[:].rearrange("p (a c) -> p a c", c=w),
                    wdram[:, :].rearrange("(a p) c -> p a c", p=128))

            def dma_x(lo, hi):
                nc.sync.dma_start(
                    xts_all[:].rearrange("p (a t) -> p a t", t=S)[:, :, lo:hi],
                    xT[:, lo:hi].rearrange("(a p) t -> p a t", p=128))

            # trigger order ~= serial service order: wq/wk then x tokens
            # 0..511 gate the first scores; wv next (v-proj), wp last.
            # load_library AFTER the weight DMAs -- it blocks the gpsimd
            # sequencer for microseconds and the broadcasts it enables
            # aren't needed until the first PV group completes.
            # all inputs on the sync HWDGE queue (gpsimd DMA is software-
            # DGE: descriptor generation alone costs ~7us per weight matrix)
            dma_w(nc.sync, wq_all, wqT, KC, CPB)
            dma_w(nc.sync, wk_all, wkT, KC, CPB)
            dma_x(0, 512)
            dma_w(nc.sync, wv_all, wvT, KC, CPB)
            dma_x(512, 1024)
            nc.gpsimd.load_library(library_config.attn)
            dma_x(1024, 1536)
            dma_x(1536, 2048)
            dma_w(nc.sync, wp_all, wpT, 2, C)

            # ---- constants ----
            onesc = small_pool.tile([128, HPC], bf16, tag="onesc")
            nc.vector.memset(onesc[:], 1.0)
            junk = small_pool.tile([128, 512], bf16, tag="junk")
            nc.vector.memset(junk[:], 0.5)
            # preload the exp table set off the critical path
            jexp = small_pool.tile([128, 16], f32, tag="jexp")
            nc.scalar.activation(jexp[:], junk[:, 0:16], EXP, scale=1.0)
            # warm the PE clock (HAM) with junk matmuls while input DMAs run
            for i in range(N_WARMUP):
                wps = psum_j.tile([128, 512], f32, tag="ppj", name="warm")
                nc.tensor.matmul(wps[:], junk[:, 0:128], junk[:],
                                 start=True, stop=True)

            # ---- persistent SBUF tiles ----
            qk_tiles = {}
            for m in range(2):
                for nm in ("q", "k"):
                    qk_tiles[(nm, m)] = qk_pool.tile(
                        [128, S], bf16, tag=f"{nm}{m}", name=f"{nm}T{m}")
            va = [va_pool.tile([128, HPC * 65], bf16, tag="va",
                               name=f"va{sc}") for sc in range(SC)]
            ot_tiles = [ot_pool.tile([128, S], bf16, tag=f"ot{m}",
                                     name=f"oT{m}") for m in range(2)]

            # ---- granule emitters (called in interleaved order below) ----
            def mark(label):
                EMIT_LOG.append((nc.next_id(), label))

            def emit_qk_proj_chunk(m, nm, n):
                mark(f"qkproj{m}{nm}{n}")
                """One 512-token chunk of the q or k projection for group m."""
                wts = wq_t if nm == "q" else wk_t
                dst = qk_tiles[(nm, m)]
                ps = psum_j.tile([128, 512], f32, tag="ppj", name="psproj")
                for kk in range(KC):
                    k = (kk + n * 2) % KC
                    nc.tensor.matmul(
                        ps[:],
                        wts[k][:, m * 128:(m + 1) * 128],
                        xts[k][:, n * 512:(n + 1) * 512],
                        start=(kk == 0), stop=(kk == KC - 1))
                nc.vector.tensor_copy(dst[:, n * 512:(n + 1) * 512], ps[:])

            def emit_v_chunk(sc):
                """One 128-token chunk of the v projection + ones column."""
                mark(f"vproj{sc}")
                t = va[sc]
                tones = t[:].rearrange("p (h x) -> p h x", x=65)[:, :, 64:65]
                nc.vector.tensor_copy(
                    tones, onesc[:].rearrange("p (h x) -> p h x", x=1))
                ps = psum_j.tile([128, CPB], f32, tag="ppj", name="psv")
                for k in range(KC):
                    nc.tensor.matmul(
                        ps[:],
                        xts[k][:, sc * 128:(sc + 1) * 128],
                        wv_t[k][:],
                        start=(k == 0), stop=(k == KC - 1))
                tv = t[:].rearrange("p (h x) -> p h x", x=65)[:, :, 0:64]
                pv = ps[:].rearrange("p (h d) -> p h d", d=64)
                nc.vector.tensor_copy(tv, pv)

            ycount = [0]

            def emit_yproj_pair(sc2):
                """Output projection + store for two 128-token chunks."""
                mark(f"yproj{sc2}")
                ys = ysb_pool.tile([128, 2 * C], bf16, tag="ysb", name="ysb")
                for i in range(2):
                    sc = sc2 + i
                    for n in range(2):
                        ps = psum_j.tile([128, 512], f32, tag="ppj",
                                         name="psy")
                        for k in range(2):
                            nc.tensor.matmul(
                                ps[:],
                                ot_tiles[k][:, sc * 128:(sc + 1) * 128],
                                wp_t[k][:, n * 512:(n + 1) * 512],
                                start=(k == 0), stop=(k == 1))
                        nc.vector.tensor_copy(
                            ys[:, i * C + n * 512:i * C + (n + 1) * 512],
                            ps[:])
                # output on the sync queue except the last pairs (scalar is
                # done with exp by then -> parallel tail drain)
                ycount[0] += 1
                q = nc.scalar if ycount[0] >= 7 else nc.sync
                q.dma_start(
                    y[sc2 * 128:(sc2 + 2) * 128, :].rearrange(
                        "(a p) c -> p a c", p=128),
                    ys[:].rearrange("p (a c) -> p a c", c=C))

            # filler queue: projection / yproj granules interleaved into the
            # attention emission so the PE never sits behind a psum slot
            # waiting for the scalar engine's exp backlog. ensure() force-
            # emits a granule whose tile a consumer is about to read --
            # emission order IS the dependency order under Tile.
            pending = {}
            order = []

            def enqueue(key, th):
                pending[key] = th
                order.append(key)

            def fill(k=1):
                while k > 0 and order:
                    th = pending.pop(order.pop(0), None)
                    if th is not None:
                        th()
                        k -= 1

            def ensure(key):
                th = pending.pop(key, None)
                if th is not None:
                    order.remove(key)
                    th()

            # ---- attention ----
            def emit_scores(m, g, kv, pvoff, w, qbase):
                """Score matmuls + exp for one (group, key-view) step.

                Returns the two exp tiles (h0, h1), each [128, 2*w] bf16
                laid out kc0|kc1 along the free dim, key on partitions.
                """
                mark(f"scores{m}g{g}kv{kv}")
                kT = qk_tiles[("k", m)]
                qT = qk_tiles[("q", m)]
                qlo = qbase + (pvoff if w == 256 else 0)
                ets = []
                for h in range(2):
                    pse = psum_e.tile([128, 2 * w], f32, tag="pse",
                                      name="pss")
                    for j in range(2):
                        kc = 2 * kv + j
                        nc.tensor.matmul(
                            pse[:, j * w:(j + 1) * w],
                            kT[64 * h:64 * (h + 1), kc * 128:(kc + 1) * 128],
                            qT[64 * h:64 * (h + 1), qlo:qlo + w],
                            start=(w == 512 or j == 0),
                            stop=(w == 512 or j == 1))
                    et = exp_pool.tile([128, 2 * w], bf16, tag="exp",
                                       name="et")
                    nc.scalar.activation(et[:], pse[:], EXP,
                                         scale=float(SCALE))
                    ets.append(et)
                return ets

            def emit_pv(pg, m, h, kv, pvoff, w, et, first, last):
                """Stream one exp tile into the [65, 512] PV psum chain."""
                mark(f"pv{m}h{h}kv{kv}")
                hh = 2 * m + h
                for j in range(2):
                    kc = 2 * kv + j
                    nc.tensor.matmul(
                        pg[:, pvoff:pvoff + w] if w == 256 else pg[:],
                        va[kc][:, hh * 65:(hh + 1) * 65],
                        et[:, j * w:(j + 1) * w],
                        start=(first and j == 0),
                        stop=(last and j == 1))

            def emit_norm(pg, m, g, h):
                """denominator row -> reciprocal -> broadcast -> normalize."""
                mark(f"norm{m}g{g}h{h}")
                rcp = nrm_pool.tile([1, 512], f32, tag="rcp", bufs=4,
                                    name="rcp")
                nc.vector.reciprocal(rcp[:], pg[64:65, :])
                bc = nrm_pool.tile([64, 512], f32, tag="bc", bufs=4,
                                   name="bc")
                nc.gpsimd.partition_broadcast(bc[:], rcp[:], channels=64)
                nc.vector.tensor_tensor(
                    ot_tiles[m][64 * h:64 * (h + 1), 512 * g:512 * (g + 1)],
                    pg[0:64, :], bc[:], MULT)

            def emit_attn_group(m, g):
                qbase, steps = _group_steps(g)
                pgs = [psum_v.tile([65, 512], f32, tag="ppv",
                                   name=f"pg{m}{g}{h}") for h in range(2)]
                prev = None
                for si, (kv, pvoff, w) in enumerate(steps):
                    ets = emit_scores(m, g, kv, pvoff, w, qbase)
                    if prev is not None:
                        pkv, ppv_off, pw, pets, pfirst = prev
                        ensure(("v", 2 * pkv))
                        ensure(("v", 2 * pkv + 1))
                        for h in range(2):
                            emit_pv(pgs[h], m, h, pkv, ppv_off, pw, pets[h],
                                    pfirst, False)
                    fill()
                    prev = (kv, pvoff, w, ets, si == 0)
                # drain last step
                pkv, ppv_off, pw, pets, pfirst = prev
                ensure(("v", 2 * pkv))
                ensure(("v", 2 * pkv + 1))
                for h in range(2):
                    emit_pv(pgs[h], m, h, pkv, ppv_off, pw, pets[h],
                            pfirst, True)
                for h in range(2):
                    emit_norm(pgs[h], m, g, h)

            # ---- emission schedule ----
            # fillers: v chunks + m=1 q/k proj, pulled between attention
            # steps (or force-emitted by ensure() right before first use).
            # Order respects DMA arrival: everything needing x tokens
            # 1024..2047 (v8+, m1 proj chunks n2/n3) goes late.
            for sc in range(0, 8):
                enqueue(("v", sc), lambda sc=sc: emit_v_chunk(sc))
            for qi in range(4):
                enqueue(("qk", qi), lambda nm="qk"[qi % 2], n=qi // 2:
                        emit_qk_proj_chunk(1, nm, n))
            for sc in range(8, 12):
                enqueue(("v", sc), lambda sc=sc: emit_v_chunk(sc))
            for qi in range(4, 8):
                enqueue(("qk", qi), lambda nm="qk"[qi % 2], n=qi // 2:
                        emit_qk_proj_chunk(1, nm, n))
            for sc in range(12, SC):
                enqueue(("v", sc), lambda sc=sc: emit_v_chunk(sc))

            # Group schedule: m=0 groups pipeline with their just-in-time
            # q/k projections (group g's queries are exactly q-chunk n=g,
            # its keys are k-chunks <= g); m=1 groups are interleaved to
            # smooth the scalar engine's exp backlog, ending on the
            # smallest group so the exp->PV->norm->yproj tail is short.
            done_m = set()
            for m, g in [(0, 0), (0, 1), (0, 2), (1, 1), (0, 3),
                         (1, 2), (1, 3), (1, 0)]:
                if m == 0:
                    emit_qk_proj_chunk(0, "q", g)
                    emit_qk_proj_chunk(0, "k", g)
                else:
                    # force m=1 q/k chunks this group reads ("qk", i):
                    # i even = q chunk i//2, i odd = k chunk i//2
                    ensure(("qk", 2 * g))
                    for n in range(g + 1):
                        ensure(("qk", 2 * n + 1))
                emit_attn_group(m, g)
                done_m.add((m, g))
                if (1 - m, g) in done_m:
                    for sc2 in (4 * g, 4 * g + 2):
                        enqueue(("y", sc2),
                                lambda sc2=sc2: emit_yproj_pair(sc2))
            while order:
                fill()

    nc.compile()
    return nc


def _get_compiled():
    if "nc" not in _compiled:
        _compiled["nc"] = build()
    return _compiled["nc"]


def make_in_maps(x, Wq, Wk, Wv, Wp):
    xf = np.asarray(x, np.float32).reshape(B, S, C)
    in_maps = []
    for c in range(N_CORES):
        b, g = divmod(c, HPC)
        hs = slice(g * CPB, (g + 1) * CPB)
        bf = ml_dtypes.bfloat16
        in_maps.append({
            "xT": np.ascontiguousarray(xf[b].T).astype(bf),
            "wqT": np.ascontiguousarray(np.asarray(Wq, np.float32)[hs].T).astype(bf),
            "wkT": np.ascontiguousarray(np.asarray(Wk, np.float32)[hs].T).astype(bf),
            "wvT": np.ascontiguousarray(np.asarray(Wv, np.float32)[hs].T).astype(bf),
            "wpT": np.ascontiguousarray(np.asarray(Wp, np.float32)[:, hs].T).astype(bf),
        })
    return in_maps


def kernel(x, Wq, Wk, Wv, Wp, bp, _trace=False, _tmpdir=None):
    global LAST_RESULTS
    from concourse import bass_utils

    nc = _get_compiled()
    in_maps = make_in_maps(x, Wq, Wk, Wv, Wp)
    kwargs = {}
    if _trace:
        kwargs = {"trace": True, "tmpdir": _tmpdir}
    res = bass_utils.run_bass_kernel_spmd(
        nc, in_maps, core_ids=list(range(N_CORES)), **kwargs)
    LAST_RESULTS = res
    yout = np.zeros((B, S, C), np.float32)
    for c in range(N_CORES):
        yout[c // HPC] += res.results[c]["y"].astype(np.float32)
    yout += np.asarray(bp, np.float32).reshape(1, 1, C)
    return yout.reshape(B, V, L, C)


# revision 38
# speedup vs baseline: 1.0355x; 1.0355x over previous
"""Block-sparse (view-causal) multi-head attention on 8 TRN2 NeuronCores.

Full inputs in, full output out. Sharding: data-parallel over batch (B=2),
tensor-parallel over heads (16 heads -> 4 per core). Each core computes its
4 heads' attention + its slice of the output projection; the host sums the
4 head-group partial projections per batch (the tensor-parallel reduce).

v2 layout: scores are computed transposed (key on partitions, query free),
exp'd on the scalar engine into SBUF, then the PV matmul is FLIPPED: the
stationary operand is V augmented with a ones column ([128 keys, 64+1]), the
exp tile is streamed at N=512. This (a) produces the attention output
directly transposed ([dh, q]) as the out-projection wants it -- no PE
transposes, (b) keeps the softmax denominator in psum row 64, and (c) turns
PV from LDWEIGHTS-bound (559 stationary loads of exp tiles) into
stream-bound. The denominator row is reciprocal'd (DVE), broadcast across
64 partitions (GpSimd, otherwise idle), and multiplied in during psum
evacuation. Adjacent causal views share key blocks, so their score/PV
streams are merged to 512-wide where possible. Exp runs on the scalar
engine from ~10us in (first score tile) to the end, overlapped with all
projection matmuls via manual emission interleaving (engines execute their
streams in order).
"""

import sys

if "/opt/trn_rl_repo" not in sys.path:
    sys.path.insert(0, "/opt/trn_rl_repo")

import numpy as np
import ml_dtypes

B, V, L, C, H = 2, 8, 256, 1024, 16
S = V * L                # 2048 tokens
DH = C // H              # 64
HPC = 4                  # heads per core
CPB = HPC * DH           # 256 channel block per core
N_CORES = 8
SCALE = DH ** -0.5       # 1/8, folded into the exp activation

_compiled = {}
LAST_RESULTS = None
EMIT_LOG = []            # (instruction-id watermark, label) for profiling
PACK_QK = True           # unused (kept for test.py compat)
SAFE_RECIP = False
SPLIT_ACT = False
N_WARMUP = 24            # junk matmuls to cover input DMA + HAM warmup


def _group_steps(g):
    """Score/PV steps for query group g (views 2g, 2g+1).

    Returns (qbase, steps) where each step is (kv, pvoff, w):
    kv = key view streamed, pvoff = column offset in the [65, 512] PV psum,
    w = query width. qbase is the global query offset of the group.
    """
    if g == 0:
        # view 0 attends view 1 only; view 1 attends view 0 only
        return 0, [(1, 0, 256), (0, 256, 256)]
    qa = 2 * g
    steps = [(kv, 0, 512) for kv in range(qa + 1)]   # shared causal blocks
    steps.append((qa + 1, 256, 256))                 # extra diag for view qa+1
    return qa * 256, steps


def build():
    import concourse.tile as tile
    from concourse import bacc, mybir, library_config

    f32 = mybir.dt.float32
    bf16 = mybir.dt.bfloat16
    EXP = mybir.ActivationFunctionType.Exp
    MULT = mybir.AluOpType.mult

    nc = bacc.Bacc("TRN2", target_bir_lowering=False, debug=False,
                   num_devices=N_CORES)
    xT = nc.dram_tensor("xT", [C, S], bf16, kind="ExternalInput").ap()
    wqT = nc.dram_tensor("wqT", [C, CPB], bf16, kind="ExternalInput").ap()
    wkT = nc.dram_tensor("wkT", [C, CPB], bf16, kind="ExternalInput").ap()
    wvT = nc.dram_tensor("wvT", [C, CPB], bf16, kind="ExternalInput").ap()
    wpT = nc.dram_tensor("wpT", [CPB, C], bf16, kind="ExternalInput").ap()
    y = nc.dram_tensor("y", [S, C], bf16, kind="ExternalOutput").ap()

    KC = C // 128        # 8 contraction chunks for the projections
    NS = S // 512        # 4 free-dim chunks for q/k projections
    SC = S // 128        # 16 sequence chunks

    with tile.TileContext(nc) as tc:
        with (
            tc.tile_pool(name="xt", bufs=1) as xt_pool,
            tc.tile_pool(name="wts", bufs=1) as w_pool,
            tc.tile_pool(name="qk", bufs=1) as qk_pool,
            tc.tile_pool(name="va", bufs=SC) as va_pool,
            tc.tile_pool(name="ot", bufs=1) as ot_pool,
            tc.tile_pool(name="exp", bufs=8) as exp_pool,
            tc.tile_pool(name="nrm", bufs=1) as nrm_pool,
            tc.tile_pool(name="small", bufs=1) as small_pool,
            tc.tile_pool(name="ysb", bufs=3) as ysb_pool,
            tc.tile_pool(name="pse", bufs=2, space="PSUM") as psum_e,
            tc.tile_pool(name="ppj", bufs=2, space="PSUM") as psum_j,
            tc.tile_pool(name="ppv", bufs=2, space="PSUM") as psum_v,
        ):
            # ---- input DMAs. A dma_start trigger costs >1us on the issuing
            # engine's sequencer, so batch everything into a handful of
            # multi-tile transfers. Weights ride gpsimd (one DMA per
            # matrix); xT rides sync in three token-range pieces so the
            # first 512 tokens (q/k chunk 0 -> first scores -> first exp)
            # land as early as possible. The scalar engine issues NO input
            # DMAs -- it must start exp'ing immediately.
            xts_all = xt_pool.tile([128, KC * S], bf16, tag="xt",
                                   name="xts_all")
            xts = [xts_all[:, k * S:(k + 1) * S] for k in range(KC)]
            wq_all = w_pool.tile([128, KC * CPB], bf16, tag="wq", name="wqa")
            wk_all = w_pool.tile([128, KC * CPB], bf16, tag="wk", name="wka")
            wv_all = w_pool.tile([128, KC * CPB], bf16, tag="wv", name="wva")
            wp_all = w_pool.tile([128, 2 * C], bf16, tag="wp", name="wpa")
            wq_t = [wq_all[:, k * CPB:(k + 1) * CPB] for k in range(KC)]
            wk_t = [wk_all[:, k * CPB:(k + 1) * CPB] for k in range(KC)]
            wv_t = [wv_all[:, k * CPB:(k + 1) * CPB] for k in range(KC)]
            wp_t = [wp_all[:, k * C:(k + 1) * C] for k in range(2)]

            def dma_w(q, wall, wdram, nt, w):
                # partition dim FIRST on the SBUF side (dep tracking
                # interprets dim0 as partitions)
                q.dma_start(
                    wall[:].rearrange("p (a c) -> p a c", c=w),
                    wdram[:, :].rearrange("(a p) c -> p a c", p=128))

            def dma_x(lo, hi):
                nc.sync.dma_start(
                    xts_all[:].rearrange("p (a t) -> p a t", t=S)[:, :, lo:hi],
                    xT[:, lo:hi].rearrange("(a p) t -> p a t", p=128))

            # trigger order ~= serial service order: wq/wk then x tokens
            # 0..511 gate the first scores; wv next (v-proj), wp last.
            # load_library AFTER the weight DMAs -- it blocks the gpsimd
            # sequencer for microseconds and the broadcasts it enables
            # aren't needed until the first PV group completes.
            # all inputs on the sync HWDGE queue (gpsimd DMA is software-
            # DGE: descriptor generation alone costs ~7us per weight matrix)
            dma_w(nc.sync, wq_all, wqT, KC, CPB)
            dma_w(nc.sync, wk_all, wkT, KC, CPB)
            dma_x(0, 1024)
            dma_w(nc.sync, wv_all, wvT, KC, CPB)
            nc.gpsimd.load_library(library_config.attn)
            dma_x(1024, 2048)
            dma_w(nc.sync, wp_all, wpT, 2, C)

            # ---- constants ----
            onesc = small_pool.tile([128, HPC], bf16, tag="onesc")
            nc.vector.memset(onesc[:], 1.0)
            junk = small_pool.tile([128, 512], bf16, tag="junk")
            nc.vector.memset(junk[:], 0.5)
            # preload the exp table set off the critical path
            jexp = small_pool.tile([128, 16], f32, tag="jexp")
            nc.scalar.activation(jexp[:], junk[:, 0:16], EXP, scale=1.0)
            # warm the PE clock (HAM) with junk matmuls while input DMAs run
            for i in range(N_WARMUP):
                wps = psum_j.tile([128, 512], f32, tag="ppj", name="warm")
                nc.tensor.matmul(wps[:], junk[:, 0:128], junk[:],
                                 start=True, stop=True)

            # ---- persistent SBUF tiles ----
            qk_tiles = {}
            for m in range(2):
                for nm in ("q", "k"):
                    qk_tiles[(nm, m)] = qk_pool.tile(
                        [128, S], bf16, tag=f"{nm}{m}", name=f"{nm}T{m}")
            va = [va_pool.tile([128, HPC * 65], bf16, tag="va",
                               name=f"va{sc}") for sc in range(SC)]
            ot_tiles = [ot_pool.tile([128, S], bf16, tag=f"ot{m}",
                                     name=f"oT{m}") for m in range(2)]

            # ---- granule emitters (called in interleaved order below) ----
            def mark(label):
                EMIT_LOG.append((nc.next_id(), label))

            def emit_qk_proj_chunk(m, nm, n):
                mark(f"qkproj{m}{nm}{n}")
                """One 512-token chunk of the q or k projection for group m."""
                wts = wq_t if nm == "q" else wk_t
                dst = qk_tiles[(nm, m)]
                ps = psum_j.tile([128, 512], f32, tag="ppj", name="psproj")
                for kk in range(KC):
                    k = (kk + n * 2) % KC
                    nc.tensor.matmul(
                        ps[:],
                        wts[k][:, m * 128:(m + 1) * 128],
                        xts[k][:, n * 512:(n + 1) * 512],
                        start=(kk == 0), stop=(kk == KC - 1))
                nc.vector.tensor_copy(dst[:, n * 512:(n + 1) * 512], ps[:])

            def emit_v_chunk(sc):
                """One 128-token chunk of the v projection + ones column."""
                mark(f"vproj{sc}")
                t = va[sc]
                tones = t[:].rearrange("p (h x) -> p h x", x=65)[:, :, 64:65]
                nc.vector.tensor_copy(
                    tones, onesc[:].rearrange("p (h x) -> p h x", x=1))
                ps = psum_j.tile([128, CPB], f32, tag="ppj", name="psv")
                for k in range(KC):
                    nc.tensor.matmul(
                        ps[:],
                        xts[k][:, sc * 128:(sc + 1) * 128],
                        wv_t[k][:],
                        start=(k == 0), stop=(k == KC - 1))
                tv = t[:].rearrange("p (h x) -> p h x", x=65)[:, :, 0:64]
                pv = ps[:].rearrange("p (h d) -> p h d", d=64)
                nc.vector.tensor_copy(tv, pv)

            ycount = [0]

            def emit_yproj_pair(sc2):
                """Output projection + store for two 128-token chunks."""
                mark(f"yproj{sc2}")
                ys = ysb_pool.tile([128, 2 * C], bf16, tag="ysb", name="ysb")
                for i in range(2):
                    sc = sc2 + i
                    for n in range(2):
                        ps = psum_j.tile([128, 512], f32, tag="ppj",
                                         name="psy")
                        for k in range(2):
                            nc.tensor.matmul(
                                ps[:],
                                ot_tiles[k][:, sc * 128:(sc + 1) * 128],
                                wp_t[k][:, n * 512:(n + 1) * 512],
                                start=(k == 0), stop=(k == 1))
                        nc.vector.tensor_copy(
                            ys[:, i * C + n * 512:i * C + (n + 1) * 512],
                            ps[:])
                # output on the sync queue except the last pairs (scalar is
                # done with exp by then -> parallel tail drain)
                ycount[0] += 1
                q = nc.scalar if ycount[0] >= 7 else nc.sync
                q.dma_start(
                    y[sc2 * 128:(sc2 + 2) * 128, :].rearrange(
                        "(a p) c -> p a c", p=128),
                    ys[:].rearrange("p (a c) -> p a c", c=C))

            # filler queue: projection / yproj granules interleaved into the
            # attention emission so the PE never sits behind a psum slot
            # waiting for the scalar engine's exp backlog. ensure() force-
            # emits a granule whose tile a consumer is about to read --
            # emission order IS the dependency order under Tile.
            pending = {}
            order = []

            def enqueue(key, th):
                pending[key] = th
                order.append(key)

            def fill(k=1):
                while k > 0 and order:
                    th = pending.pop(order.pop(0), None)
                    if th is not None:
                        th()
                        k -= 1

            def ensure(key):
                th = pending.pop(key, None)
                if th is not None:
                    order.remove(key)
                    th()

            # ---- attention ----
            def emit_scores(m, g, kv, pvoff, w, qbase):
                """Score matmuls + exp for one (group, key-view) step.

                Returns the two exp tiles (h0, h1), each [128, 2*w] bf16
                laid out kc0|kc1 along the free dim, key on partitions.
                """
                mark(f"scores{m}g{g}kv{kv}")
                kT = qk_tiles[("k", m)]
                qT = qk_tiles[("q", m)]
                qlo = qbase + (pvoff if w == 256 else 0)
                ets = []
                for h in range(2):
                    pse = psum_e.tile([128, 2 * w], f32, tag="pse",
                                      name="pss")
                    for j in range(2):
                        kc = 2 * kv + j
                        nc.tensor.matmul(
                            pse[:, j * w:(j + 1) * w],
                            kT[64 * h:64 * (h + 1), kc * 128:(kc + 1) * 128],
                            qT[64 * h:64 * (h + 1), qlo:qlo + w],
                            start=(w == 512 or j == 0),
                            stop=(w == 512 or j == 1))
                    et = exp_pool.tile([128, 2 * w], bf16, tag="exp",
                                       name="et")
                    nc.scalar.activation(et[:], pse[:], EXP,
                                         scale=float(SCALE))
                    ets.append(et)
                return ets

            def emit_pv(pg, m, h, kv, pvoff, w, et, first, last):
                """Stream one exp tile into the [65, 512] PV psum chain."""
                mark(f"pv{m}h{h}kv{kv}")
                hh = 2 * m + h
                for j in range(2):
                    kc = 2 * kv + j
                    nc.tensor.matmul(
                        pg[:, pvoff:pvoff + w] if w == 256 else pg[:],
                        va[kc][:, hh * 65:(hh + 1) * 65],
                        et[:, j * w:(j + 1) * w],
                        start=(first and j == 0),
                        stop=(last and j == 1))

            def emit_norm(pg, m, g, h):
                """denominator row -> reciprocal -> broadcast -> normalize."""
                mark(f"norm{m}g{g}h{h}")
                # single-lane [1,512] exact reciprocal costs 3.3us on the
                # DVE, so: evacuate the denominator row to SBUF, broadcast
                # the RAW row across 64 partitions (gpsimd), then run the
                # 1-pass approx reciprocal on the standard [64,512] shape
                # (18 bits is far more than the softmax denominator needs).
                den = nrm_pool.tile([1, 512], f32, tag="den", bufs=4,
                                    name="den")
                nc.vector.tensor_copy(den[:], pg[64:65, :])
                bc = nrm_pool.tile([64, 512], f32, tag="bc", bufs=4,
                                   name="bc")
                nc.gpsimd.partition_broadcast(bc[:], den[:], channels=64)
                rcp = nrm_pool.tile([64, 512], f32, tag="rcp", bufs=4,
                                    name="rcp")
                nc.vector.reciprocal_approx_fast(rcp[:], bc[:])
                nc.vector.tensor_tensor(
                    ot_tiles[m][64 * h:64 * (h + 1), 512 * g:512 * (g + 1)],
                    pg[0:64, :], rcp[:], MULT)

            def emit_attn_group(m, g):
                qbase, steps = _group_steps(g)
                pgs = [psum_v.tile([65, 512], f32, tag="ppv",
                                   name=f"pg{m}{g}{h}") for h in range(2)]
                prev = None
                for si, (kv, pvoff, w) in enumerate(steps):
                    ets = emit_scores(m, g, kv, pvoff, w, qbase)
                    if prev is not None:
                        pkv, ppv_off, pw, pets, pfirst = prev
                        ensure(("v", 2 * pkv))
                        ensure(("v", 2 * pkv + 1))
                        for h in range(2):
                            emit_pv(pgs[h], m, h, pkv, ppv_off, pw, pets[h],
                                    pfirst, False)
                    fill()
                    prev = (kv, pvoff, w, ets, si == 0)
                # drain last step
                pkv, ppv_off, pw, pets, pfirst = prev
                ensure(("v", 2 * pkv))
                ensure(("v", 2 * pkv + 1))
                for h in range(2):
                    emit_pv(pgs[h], m, h, pkv, ppv_off, pw, pets[h],
                            pfirst, True)
                for h in range(2):
                    emit_norm(pgs[h], m, g, h)

            # ---- emission schedule ----
            # fillers: v chunks + m=1 q/k proj, pulled between attention
            # steps (or force-emitted by ensure() right before first use).
            # Order respects DMA arrival: everything needing x tokens
            # 1024..2047 (v8+, m1 proj chunks n2/n3) goes late.
            for sc in range(0, 8):
                enqueue(("v", sc), lambda sc=sc: emit_v_chunk(sc))
            for qi in range(4):
                enqueue(("qk", qi), lambda nm="qk"[qi % 2], n=qi // 2:
                        emit_qk_proj_chunk(1, nm, n))
            for sc in range(8, 12):
                enqueue(("v", sc), lambda sc=sc: emit_v_chunk(sc))
            for qi in range(4, 8):
                enqueue(("qk", qi), lambda nm="qk"[qi % 2], n=qi // 2:
                        emit_qk_proj_chunk(1, nm, n))
            for sc in range(12, SC):
                enqueue(("v", sc), lambda sc=sc: emit_v_chunk(sc))

            # Group schedule: m=0 groups pipeline with their just-in-time
            # q/k projections (group g's queries are exactly q-chunk n=g,
            # its keys are k-chunks <= g); m=1 groups are interleaved to
            # smooth the scalar engine's exp backlog, ending on the
            # smallest group so the exp->PV->norm->yproj tail is short.
            done_m = set()
            for m, g in [(0, 0), (0, 1), (0, 2), (1, 1), (0, 3),
                         (1, 2), (1, 3), (1, 0)]:
                if m == 0:
                    emit_qk_proj_chunk(0, "q", g)
                    emit_qk_proj_chunk(0, "k", g)
                else:
                    # force m=1 q/k chunks this group reads ("qk", i):
                    # i even = q chunk i//2, i odd = k chunk i//2
                    ensure(("qk", 2 * g))
                    for n in range(g + 1):
                        ensure(("qk", 2 * n + 1))
                emit_attn_group(m, g)
                done_m.add((m, g))
                if (1 - m, g) in done_m:
                    for sc2 in (4 * g, 4 * g + 2):
                        enqueue(("y", sc2),
                                lambda sc2=sc2: emit_yproj_pair(sc2))
            while order:
                fill()

    nc.compile()
    return nc


def _get_compiled():
    if "nc" not in _compiled:
        _compiled["nc"] = build()
    return _compiled["nc"]


def make_in_maps(x, Wq, Wk, Wv, Wp):
    xf = np.asarray(x, np.float32).reshape(B, S, C)
    in_maps = []
    for c in range(N_CORES):
        b, g = divmod(c, HPC)
        hs = slice(g * CPB, (g + 1) * CPB)
        bf = ml_dtypes.bfloat16
        in_maps.append({
            "xT": np.ascontiguousarray(xf[b].T).astype(bf),
            "wqT": np.ascontiguousarray(np.asarray(Wq, np.float32)[hs].T).astype(bf),
            "wkT": np.ascontiguousarray(np.asarray(Wk, np.float32)[hs].T).astype(bf),
            "wvT": np.ascontiguousarray(np.asarray(Wv, np.float32)[hs].T).astype(bf),
            "wpT": np.ascontiguousarray(np.asarray(Wp, np.float32)[:, hs].T).astype(bf),
        })
    return in_maps


def kernel(x, Wq, Wk, Wv, Wp, bp, _trace=False, _tmpdir=None):
    global LAST_RESULTS
    from concourse import bass_utils

    nc = _get_compiled()
    in_maps = make_in_maps(x, Wq, Wk, Wv, Wp)
    kwargs = {}
    if _trace:
        kwargs = {"trace": True, "tmpdir": _tmpdir}
    res = bass_utils.run_bass_kernel_spmd(
        nc, in_maps, core_ids=list(range(N_CORES)), **kwargs)
    LAST_RESULTS = res
    yout = np.zeros((B, S, C), np.float32)
    for c in range(N_CORES):
        yout[c // HPC] += res.results[c]["y"].astype(np.float32)
    yout += np.asarray(bp, np.float32).reshape(1, 1, C)
    return yout.reshape(B, V, L, C)


# revision 41
# speedup vs baseline: 1.4085x; 1.3602x over previous
"""Block-sparse (view-causal) multi-head attention on 8 TRN2 NeuronCores.

Full inputs in, full output out. Sharding: data-parallel over batch (B=2),
tensor-parallel over heads (16 heads -> 4 per core). Each core computes its
4 heads' attention + its slice of the output projection; the host sums the
4 head-group partial projections per batch (the tensor-parallel reduce).

v5 = v1's compute structure + v2's scheduling.

Compute (per head-pair m, per query view qv): scores transposed (key on
partitions) with h row-group-packed K=64 matmuls; exp on the scalar engine
(psum -> SBUF bf16); PV natural (stationary = exp tile [128k,128q], stream =
V augmented with a ones column) so the softmax denominator lands as a psum
COLUMN -> partition-parallel reciprocal + tensor_scalar normalize on the
DVE; attention out is PE-transposed back to [channel, token] for the output
projection.

Scheduling: all input DMA batched into a handful of multi-tile transfers on
the sync HWDGE queue (a dma_start trigger costs >1us of issuing-engine
sequencer time, and gpsimd software-DGE costs ~7us per matrix); the scalar
engine issues nothing but exp, starting ~10us in (q/k chunk 0 is projected
just-in-time from the first xT piece); v-projection chunks, m=1 q/k
projection chunks, transpose+output-projection granules ride a filler queue
interleaved between attention steps so the PE never idles behind the exp
backlog; output DMA is batched in 2-chunk pieces; the schedule ends on the
smallest query views to keep the exp->PV->transpose->yproj tail short.
"""

import sys

if "/opt/trn_rl_repo" not in sys.path:
    sys.path.insert(0, "/opt/trn_rl_repo")

import numpy as np
import ml_dtypes

B, V, L, C, H = 2, 8, 256, 1024, 16
S = V * L                # 2048 tokens
DH = C // H              # 64
HPC = 4                  # heads per core
CPB = HPC * DH           # 256 channel block per core
N_CORES = 8
SCALE = DH ** -0.5       # 1/8, folded into the exp activation

_compiled = {}
LAST_RESULTS = None
EMIT_LOG = []            # (instruction-id watermark, label) for profiling
PACK_QK = True           # unused (kept for test.py compat)
SAFE_RECIP = False
SPLIT_ACT = False
N_WARMUP = 24            # junk matmuls to cover input DMA + HAM warmup


def _allowed(qv):
    """View-level mask row: views 0/1 cross-attend only; views >=2 block-causal."""
    if qv == 0:
        return [1]
    if qv == 1:
        return [0]
    return list(range(qv + 1))


def build():
    import concourse.tile as tile
    from concourse import bacc, mybir
    from concourse.masks import make_identity

    f32 = mybir.dt.float32
    bf16 = mybir.dt.bfloat16
    EXP = mybir.ActivationFunctionType.Exp

    nc = bacc.Bacc("TRN2", target_bir_lowering=False, debug=False,
                   num_devices=N_CORES)
    xT = nc.dram_tensor("xT", [C, S], bf16, kind="ExternalInput").ap()
    wqT = nc.dram_tensor("wqT", [C, CPB], bf16, kind="ExternalInput").ap()
    wkT = nc.dram_tensor("wkT", [C, CPB], bf16, kind="ExternalInput").ap()
    wvT = nc.dram_tensor("wvT", [C, CPB], bf16, kind="ExternalInput").ap()
    wpT = nc.dram_tensor("wpT", [CPB, C], bf16, kind="ExternalInput").ap()
    y = nc.dram_tensor("y", [S, C], bf16, kind="ExternalOutput").ap()

    KC = C // 128        # 8 contraction chunks for the projections
    NS = S // 512        # 4 free-dim chunks for q/k projections
    SC = S // 128        # 16 sequence chunks

    with tile.TileContext(nc) as tc:
        with (
            tc.tile_pool(name="xt", bufs=1) as xt_pool,
            tc.tile_pool(name="wts", bufs=1) as w_pool,
            tc.tile_pool(name="qk", bufs=1) as qk_pool,
            tc.tile_pool(name="va", bufs=SC) as va_pool,
            tc.tile_pool(name="ot", bufs=1) as ot_pool,
            tc.tile_pool(name="exp", bufs=10) as exp_pool,
            tc.tile_pool(name="small", bufs=1) as small_pool,
            tc.tile_pool(name="ysb", bufs=3) as ysb_pool,
            tc.tile_pool(name="pse", bufs=2, space="PSUM") as psum_e,
            tc.tile_pool(name="ppj", bufs=2, space="PSUM") as psum_j,
            tc.tile_pool(name="ppv", bufs=2, space="PSUM") as psum_v,
        ):
            # ---- input DMAs: batched multi-tile transfers, all on the
            # sync HWDGE queue, ordered so wq/wk + the first xT piece gate
            # nothing downstream for long ----
            xts_all = xt_pool.tile([128, KC * S], bf16, tag="xt",
                                   name="xts_all")
            xts = [xts_all[:, k * S:(k + 1) * S] for k in range(KC)]
            wq_all = w_pool.tile([128, KC * CPB], bf16, tag="wq", name="wqa")
            wk_all = w_pool.tile([128, KC * CPB], bf16, tag="wk", name="wka")
            wv_all = w_pool.tile([128, KC * CPB], bf16, tag="wv", name="wva")
            wp_all = w_pool.tile([128, 2 * C], bf16, tag="wp", name="wpa")
            wq_t = [wq_all[:, k * CPB:(k + 1) * CPB] for k in range(KC)]
            wk_t = [wk_all[:, k * CPB:(k + 1) * CPB] for k in range(KC)]
            wv_t = [wv_all[:, k * CPB:(k + 1) * CPB] for k in range(KC)]
            wp_t = [wp_all[:, k * C:(k + 1) * C] for k in range(2)]

            def dma_w(wall, wdram, w):
                nc.sync.dma_start(
                    wall[:].rearrange("p (a c) -> p a c", c=w),
                    wdram[:, :].rearrange("(a p) c -> p a c", p=128))

            def dma_x(lo, hi):
                nc.sync.dma_start(
                    xts_all[:].rearrange("p (a t) -> p a t", t=S)[:, :, lo:hi],
                    xT[:, lo:hi].rearrange("(a p) t -> p a t", p=128))

            dma_w(wq_all, wqT, CPB)
            dma_w(wk_all, wkT, CPB)
            dma_x(0, 1024)
            dma_w(wv_all, wvT, CPB)
            dma_x(1024, 2048)
            dma_w(wp_all, wpT, C)

            # ---- constants ----
            onesc = small_pool.tile([128, HPC], bf16, tag="onesc")
            nc.vector.memset(onesc[:], 1.0)
            junk = small_pool.tile([128, 512], bf16, tag="junk")
            nc.vector.memset(junk[:], 0.5)
            ident = small_pool.tile([128, 128], bf16, tag="ident")
            make_identity(nc, ident[:])
            # preload the exp table set off the critical path
            jexp = small_pool.tile([128, 16], f32, tag="jexp")
            nc.scalar.activation(jexp[:], junk[:, 0:16], EXP, scale=1.0)
            # warm the PE clock (HAM) with junk matmuls while input DMAs run
            for i in range(N_WARMUP):
                wps = psum_j.tile([128, 512], f32, tag="ppj", name="warm")
                nc.tensor.matmul(wps[:], junk[:, 0:128], junk[:],
                                 start=True, stop=True)

            # ---- persistent SBUF tiles ----
            qk_tiles = {}
            for m in range(2):
                for nm in ("q", "k"):
                    qk_tiles[(nm, m)] = qk_pool.tile(
                        [128, S], bf16, tag=f"{nm}{m}", name=f"{nm}T{m}")
            va = [va_pool.tile([128, HPC * 65], bf16, tag="va",
                               name=f"va{sc}") for sc in range(SC)]
            on_tiles = [small_pool.tile([128, CPB], bf16, tag="on", bufs=SC,
                                        name=f"on{sc}") for sc in range(SC)]
            ot_tiles = [ot_pool.tile([128, S], bf16, tag=f"ot{m}",
                                     name=f"oT{m}") for m in range(2)]

            def mark(label):
                EMIT_LOG.append((nc.next_id(), label))

            # ---- granule emitters ----
            def emit_qk_proj_chunk(m, nm, n):
                mark(f"qkproj{m}{nm}{n}")
                wts = wq_t if nm == "q" else wk_t
                dst = qk_tiles[(nm, m)]
                ps = psum_j.tile([128, 512], f32, tag="ppj", name="psproj")
                for kk in range(KC):
                    k = (kk + n * 2) % KC
                    nc.tensor.matmul(
                        ps[:],
                        wts[k][:, m * 128:(m + 1) * 128],
                        xts[k][:, n * 512:(n + 1) * 512],
                        start=(kk == 0), stop=(kk == KC - 1))
                nc.vector.tensor_copy(dst[:, n * 512:(n + 1) * 512], ps[:])

            def emit_v_chunk(sc):
                mark(f"vproj{sc}")
                t = va[sc]
                tones = t[:].rearrange("p (h x) -> p h x", x=65)[:, :, 64:65]
                nc.vector.tensor_copy(
                    tones, onesc[:].rearrange("p (h x) -> p h x", x=1))
                ps = psum_j.tile([128, CPB], f32, tag="ppj", name="psv")
                for k in range(KC):
                    nc.tensor.matmul(
                        ps[:],
                        xts[k][:, sc * 128:(sc + 1) * 128],
                        wv_t[k][:],
                        start=(k == 0), stop=(k == KC - 1))
                tv = t[:].rearrange("p (h x) -> p h x", x=65)[:, :, 0:64]
                pv = ps[:].rearrange("p (h d) -> p h d", d=64)
                nc.vector.tensor_copy(tv, pv)

            # filler machinery (see v2): emission order IS dependency order
            pending = {}
            order = []

            def enqueue(key, th):
                pending[key] = th
                order.append(key)

            def fill(k=1):
                while k > 0 and order:
                    th = pending.pop(order.pop(0), None)
                    if th is not None:
                        th()
                        k -= 1

            def ensure(key):
                th = pending.pop(key, None)
                if th is not None:
                    order.remove(key)
                    th()

            # ---- attention (v1 compute) ----
            def emit_scores(m, qv):
                """Transposed scores + exp for one query view.

                Returns [(kv, et)] where et is [128 keys, 1024] bf16 laid
                out (2h+j)*256 columns (h = head in pair, j = key chunk).
                """
                kT = qk_tiles[("k", m)]
                qT = qk_tiles[("q", m)]
                qs = slice(qv * 256, (qv + 1) * 256)
                ets = []
                for kv in _allowed(qv):
                    mark(f"scores{m}v{qv}kv{kv}")
                    pss = psum_e.tile([128, 1024], f32, tag="pse",
                                      name="pss")
                    for j in range(2):
                        kc = 2 * kv + j
                        for h in range(2):   # h inner: row groups alternate
                            nc.tensor.matmul(
                                pss[:, (2 * h + j) * 256:
                                    (2 * h + j + 1) * 256],
                                kT[64 * h:64 * (h + 1),
                                   kc * 128:(kc + 1) * 128],
                                qT[64 * h:64 * (h + 1), qs],
                                start=True, stop=True)
                    et = exp_pool.tile([128, 1024], bf16, tag="exp",
                                       name="et")
                    nc.scalar.activation(et[:], pss[:], EXP,
                                         scale=float(SCALE))
                    ets.append((kv, et))
                    fill()
                return ets

            def emit_pv(m, qv, ets):
                """Natural-layout PV + partition-parallel normalize."""
                kvs = _allowed(qv)
                rp = small_pool.tile([128, 4], f32, tag="rp", bufs=4,
                                     name="rp")
                for h in range(2):
                    hh = 2 * m + h
                    for qc in range(2):
                        mark(f"pv{m}v{qv}h{h}q{qc}")
                        g = 2 * h + qc
                        pg = psum_v.tile([128, 65], f32, tag="ppv",
                                         name=f"pg{g}")
                        for i, (kv, et) in enumerate(ets):
                            for j in range(2):
                                kc = 2 * kv + j
                                nc.tensor.matmul(
                                    pg[:],
                                    et[:, (2 * h + j) * 256 + qc * 128:
                                       (2 * h + j) * 256 + qc * 128 + 128],
                                    va[kc][:, hh * 65:(hh + 1) * 65],
                                    start=(i == 0 and j == 0),
                                    stop=(i == len(kvs) - 1 and j == 1))
                        sc = qv * 2 + qc
                        nc.vector.reciprocal(rp[:, g:g + 1], pg[:, 64:65])
                        nc.vector.tensor_scalar_mul(
                            on_tiles[sc][:, hh * 64:(hh + 1) * 64],
                            pg[:, 0:64],
                            rp[:, g:g + 1])

            def emit_trans_yproj(qv):
                """Transpose both m's attention out for views qv's two
                token chunks, then project + store them."""
                mark(f"typroj{qv}")
                ys = ysb_pool.tile([128, 2 * C], bf16, tag="ysb", name="ysb")
                for i in range(2):
                    sc = 2 * qv + i
                    for half in range(2):
                        pt = psum_j.tile([128, 128], bf16, tag="ppj",
                                         name="pt")
                        nc.tensor.transpose(
                            pt[:],
                            on_tiles[sc][:, half * 128:(half + 1) * 128],
                            ident[:])
                        nc.vector.tensor_copy(
                            ot_tiles[half][:, sc * 128:(sc + 1) * 128],
                            pt[:])
                    for n in range(2):
                        ps = psum_j.tile([128, 512], f32, tag="ppj",
                                         name="psy")
                        for k in range(2):
                            nc.tensor.matmul(
                                ps[:],
                                ot_tiles[k][:, sc * 128:(sc + 1) * 128],
                                wp_t[k][:, n * 512:(n + 1) * 512],
                                start=(k == 0), stop=(k == 1))
                        nc.vector.tensor_copy(
                            ys[:, i * C + n * 512:i * C + (n + 1) * 512],
                            ps[:])
                ycount[0] += 1
                q = nc.scalar if ycount[0] >= 7 else nc.sync
                q.dma_start(
                    y[2 * qv * 128:(2 * qv + 2) * 128, :].rearrange(
                        "(a p) c -> p a c", p=128),
                    ys[:].rearrange("p (a c) -> p a c", c=C))

            ycount = [0]

            def emit_attn(m, qv):
                ets = emit_scores(m, qv)
                fill()
                emit_pv(m, qv, ets)

            # ---- fillers: v chunks + m=1 q/k projections ----
            for sc in range(0, 8):
                enqueue(("v", sc), lambda sc=sc: emit_v_chunk(sc))
            for qi in range(4):
                enqueue(("qk", qi), lambda nm="qk"[qi % 2], n=qi // 2:
                        emit_qk_proj_chunk(1, nm, n))
            for sc in range(8, 12):
                enqueue(("v", sc), lambda sc=sc: emit_v_chunk(sc))
            for qi in range(4, 8):
                enqueue(("qk", qi), lambda nm="qk"[qi % 2], n=qi // 2:
                        emit_qk_proj_chunk(1, nm, n))
            for sc in range(12, SC):
                enqueue(("v", sc), lambda sc=sc: emit_v_chunk(sc))

            def ensure_va(qv):
                for kv in _allowed(qv):
                    ensure(("v", 2 * kv))
                    ensure(("v", 2 * kv + 1))

            # ---- schedule ----
            # m=0: project q/k chunk n just-in-time, then the two views
            # whose queries live in chunk n
            for n in range(NS):
                emit_qk_proj_chunk(0, "q", n)
                emit_qk_proj_chunk(0, "k", n)
                for qv in (2 * n, 2 * n + 1):
                    ensure_va(qv)
                    emit_attn(0, qv)
                    fill()
            # m=1: big views first, tiny views (0,1) last for a short tail
            for qv in (2, 3, 4, 5, 6, 7, 1, 0):
                ensure(("qk", 2 * (qv // 2)))       # q chunk qv//2
                for n2 in range(qv // 2 + 1):
                    ensure(("qk", 2 * n2 + 1))      # k chunks <= qv//2
                ensure_va(qv)
                emit_attn(1, qv)
                fill()
                enqueue(("ty", qv), lambda qv=qv: emit_trans_yproj(qv))
            while order:
                fill()

    nc.compile()
    return nc


def _get_compiled():
    if "nc" not in _compiled:
        _compiled["nc"] = build()
    return _compiled["nc"]


def make_in_maps(x, Wq, Wk, Wv, Wp):
    xf = np.asarray(x, np.float32).reshape(B, S, C)
    in_maps = []
    for c in range(N_CORES):
        b, g = divmod(c, HPC)
        hs = slice(g * CPB, (g + 1) * CPB)
        bf = ml_dtypes.bfloat16
        in_maps.append({
            "xT": np.ascontiguousarray(xf[b].T).astype(bf),
            "wqT": np.ascontiguousarray(np.asarray(Wq, np.float32)[hs].T).astype(bf),
            "wkT": np.ascontiguousarray(np.asarray(Wk, np.float32)[hs].T).astype(bf),
            "wvT": np.ascontiguousarray(np.asarray(Wv, np.float32)[hs].T).astype(bf),
            "wpT": np.ascontiguousarray(np.asarray(Wp, np.float32)[:, hs].T).astype(bf),
        })
    return in_maps


def kernel(x, Wq, Wk, Wv, Wp, bp, _trace=False, _tmpdir=None):
    global LAST_RESULTS
    from concourse import bass_utils

    nc = _get_compiled()
    in_maps = make_in_maps(x, Wq, Wk, Wv, Wp)
    kwargs = {}
    if _trace:
        kwargs = {"trace": True, "tmpdir": _tmpdir}
    res = bass_utils.run_bass_kernel_spmd(
        nc, in_maps, core_ids=list(range(N_CORES)), **kwargs)
    LAST_RESULTS = res
    yout = np.zeros((B, S, C), np.float32)
    for c in range(N_CORES):
        yout[c // HPC] += res.results[c]["y"].astype(np.float32)
    yout += np.asarray(bp, np.float32).reshape(1, 1, C)
    return yout.reshape(B, V, L, C)


# revision 50
# speedup vs baseline: 1.4724x; 1.0453x over previous
"""Block-sparse (view-causal) multi-head attention on 8 TRN2 NeuronCores.

Full inputs in, full output out. Sharding: data-parallel over batch (B=2),
tensor-parallel over heads (16 heads -> 4 per core). Each core computes its
4 heads' attention + its slice of the output projection; the host sums the
4 head-group partial projections per batch (the tensor-parallel reduce).

v5 = v1's compute structure + v2's scheduling.

Compute (per head-pair m, per query view qv): scores transposed (key on
partitions) with h row-group-packed K=64 matmuls; exp on the scalar engine
(psum -> SBUF bf16); PV natural (stationary = exp tile [128k,128q], stream =
V augmented with a ones column) so the softmax denominator lands as a psum
COLUMN -> partition-parallel reciprocal + tensor_scalar normalize on the
DVE; attention out is PE-transposed back to [channel, token] for the output
projection.

Scheduling: all input DMA batched into a handful of multi-tile transfers on
the sync HWDGE queue (a dma_start trigger costs >1us of issuing-engine
sequencer time, and gpsimd software-DGE costs ~7us per matrix); the scalar
engine issues nothing but exp, starting ~10us in (q/k chunk 0 is projected
just-in-time from the first xT piece); v-projection chunks, m=1 q/k
projection chunks, transpose+output-projection granules ride a filler queue
interleaved between attention steps so the PE never idles behind the exp
backlog; output DMA is batched in 2-chunk pieces; the schedule ends on the
smallest query views to keep the exp->PV->transpose->yproj tail short.
"""

import sys

if "/opt/trn_rl_repo" not in sys.path:
    sys.path.insert(0, "/opt/trn_rl_repo")

import numpy as np
import ml_dtypes

B, V, L, C, H = 2, 8, 256, 1024, 16
S = V * L                # 2048 tokens
DH = C // H              # 64
HPC = 4                  # heads per core
CPB = HPC * DH           # 256 channel block per core
N_CORES = 8
SCALE = DH ** -0.5       # 1/8, folded into the exp activation

_compiled = {}
LAST_RESULTS = None
EMIT_LOG = []            # (instruction-id watermark, label) for profiling
PACK_QK = True           # unused (kept for test.py compat)
SAFE_RECIP = False
SPLIT_ACT = False
N_WARMUP = 24            # junk matmuls to cover input DMA + HAM warmup


def _allowed(qv):
    """View-level mask row: views 0/1 cross-attend only; views >=2 block-causal."""
    if qv == 0:
        return [1]
    if qv == 1:
        return [0]
    return list(range(qv + 1))


def build():
    import concourse.tile as tile
    from concourse import bacc, mybir
    from concourse.masks import make_identity

    f32 = mybir.dt.float32
    bf16 = mybir.dt.bfloat16
    EXP = mybir.ActivationFunctionType.Exp

    nc = bacc.Bacc("TRN2", target_bir_lowering=False, debug=False,
                   num_devices=N_CORES)
    xT = nc.dram_tensor("xT", [C, S], bf16, kind="ExternalInput").ap()
    wqT = nc.dram_tensor("wqT", [C, CPB], bf16, kind="ExternalInput").ap()
    wkT = nc.dram_tensor("wkT", [C, CPB], bf16, kind="ExternalInput").ap()
    wvT = nc.dram_tensor("wvT", [C, CPB], bf16, kind="ExternalInput").ap()
    wpT = nc.dram_tensor("wpT", [CPB, C], bf16, kind="ExternalInput").ap()
    y = nc.dram_tensor("y", [S, C], bf16, kind="ExternalOutput").ap()

    KC = C // 128        # 8 contraction chunks for the projections
    NS = S // 512        # 4 free-dim chunks for q/k projections
    SC = S // 128        # 16 sequence chunks

    with tile.TileContext(nc) as tc:
        with (
            tc.tile_pool(name="xt", bufs=1) as xt_pool,
            tc.tile_pool(name="wts", bufs=1) as w_pool,
            tc.tile_pool(name="qk", bufs=1) as qk_pool,
            tc.tile_pool(name="va", bufs=SC) as va_pool,
            tc.tile_pool(name="ot", bufs=1) as ot_pool,
            tc.tile_pool(name="exp", bufs=10) as exp_pool,
            tc.tile_pool(name="small", bufs=1) as small_pool,
            tc.tile_pool(name="ysb", bufs=3) as ysb_pool,
            tc.tile_pool(name="pse", bufs=2, space="PSUM") as psum_e,
            tc.tile_pool(name="ppj", bufs=2, space="PSUM") as psum_j,
            tc.tile_pool(name="ppv", bufs=2, space="PSUM") as psum_v,
        ):
            # ---- input DMAs: batched multi-tile transfers, all on the
            # sync HWDGE queue, ordered so wq/wk + the first xT piece gate
            # nothing downstream for long ----
            xts_all = xt_pool.tile([128, KC * S], bf16, tag="xt",
                                   name="xts_all")
            xts = [xts_all[:, k * S:(k + 1) * S] for k in range(KC)]
            wq_all = w_pool.tile([128, KC * CPB], bf16, tag="wq", name="wqa")
            wk_all = w_pool.tile([128, KC * CPB], bf16, tag="wk", name="wka")
            wv_all = w_pool.tile([128, KC * CPB], bf16, tag="wv", name="wva")
            wp_all = w_pool.tile([128, 2 * C], bf16, tag="wp", name="wpa")
            wq_t = [wq_all[:, k * CPB:(k + 1) * CPB] for k in range(KC)]
            wk_t = [wk_all[:, k * CPB:(k + 1) * CPB] for k in range(KC)]
            wv_t = [wv_all[:, k * CPB:(k + 1) * CPB] for k in range(KC)]
            wp_t = [wp_all[:, k * C:(k + 1) * C] for k in range(2)]

            def dma_w(wall, wdram, w):
                nc.sync.dma_start(
                    wall[:].rearrange("p (a c) -> p a c", c=w),
                    wdram[:, :].rearrange("(a p) c -> p a c", p=128))

            def dma_x(lo, hi):
                nc.sync.dma_start(
                    xts_all[:].rearrange("p (a t) -> p a t", t=S)[:, :, lo:hi],
                    xT[:, lo:hi].rearrange("(a p) t -> p a t", p=128))

            dma_w(wq_all, wqT, CPB)
            dma_w(wk_all, wkT, CPB)
            dma_x(0, 512)
            dma_w(wv_all, wvT, CPB)
            dma_x(512, 1024)
            dma_x(1024, 2048)
            dma_w(wp_all, wpT, C)

            # ---- constants ----
            onesc = small_pool.tile([128, HPC], bf16, tag="onesc")
            nc.vector.memset(onesc[:], 1.0)
            junk = small_pool.tile([128, 512], bf16, tag="junk")
            nc.vector.memset(junk[:], 0.5)
            ident = small_pool.tile([128, 128], bf16, tag="ident")
            make_identity(nc, ident[:])
            # preload the exp table set off the critical path
            jexp = small_pool.tile([128, 16], f32, tag="jexp")
            nc.scalar.activation(jexp[:], junk[:, 0:16], EXP, scale=1.0)
            # warm the PE clock (HAM) with junk matmuls while input DMAs run
            for i in range(N_WARMUP):
                wps = psum_j.tile([128, 512], f32, tag="ppj", name="warm")
                nc.tensor.matmul(wps[:], junk[:, 0:128], junk[:],
                                 start=True, stop=True)

            # ---- persistent SBUF tiles ----
            qk_tiles = {}
            for m in range(2):
                for nm in ("q", "k"):
                    qk_tiles[(nm, m)] = qk_pool.tile(
                        [128, S], bf16, tag=f"{nm}{m}", name=f"{nm}T{m}")
            va = [va_pool.tile([128, HPC * 65], bf16, tag="va",
                               name=f"va{sc}") for sc in range(SC)]
            on_tiles = [small_pool.tile([128, CPB], bf16, tag="on", bufs=SC,
                                        name=f"on{sc}") for sc in range(SC)]
            ot_tiles = [ot_pool.tile([128, S], bf16, tag=f"ot{m}",
                                     name=f"oT{m}") for m in range(2)]

            def mark(label):
                EMIT_LOG.append((nc.next_id(), label))

            # ---- granule emitters ----
            def emit_qk_proj_chunk(m, nm, n):
                mark(f"qkproj{m}{nm}{n}")
                wts = wq_t if nm == "q" else wk_t
                dst = qk_tiles[(nm, m)]
                ps = psum_j.tile([128, 512], f32, tag="ppj", name="psproj")
                for kk in range(KC):
                    k = (kk + n * 2) % KC
                    nc.tensor.matmul(
                        ps[:],
                        wts[k][:, m * 128:(m + 1) * 128],
                        xts[k][:, n * 512:(n + 1) * 512],
                        start=(kk == 0), stop=(kk == KC - 1))
                nc.vector.tensor_copy(dst[:, n * 512:(n + 1) * 512], ps[:])

            def emit_v_chunk(sc):
                mark(f"vproj{sc}")
                t = va[sc]
                tones = t[:].rearrange("p (h x) -> p h x", x=65)[:, :, 64:65]
                nc.vector.tensor_copy(
                    tones, onesc[:].rearrange("p (h x) -> p h x", x=1))
                ps = psum_j.tile([128, CPB], f32, tag="ppj", name="psv")
                for k in range(KC):
                    nc.tensor.matmul(
                        ps[:],
                        xts[k][:, sc * 128:(sc + 1) * 128],
                        wv_t[k][:],
                        start=(k == 0), stop=(k == KC - 1))
                tv = t[:].rearrange("p (h x) -> p h x", x=65)[:, :, 0:64]
                pv = ps[:].rearrange("p (h d) -> p h d", d=64)
                nc.vector.tensor_copy(tv, pv)

            # filler machinery (see v2): emission order IS dependency order
            pending = {}
            order = []

            def enqueue(key, th):
                pending[key] = th
                order.append(key)

            def fill(k=1):
                while k > 0 and order:
                    th = pending.pop(order.pop(0), None)
                    if th is not None:
                        th()
                        k -= 1

            def ensure(key):
                th = pending.pop(key, None)
                if th is not None:
                    order.remove(key)
                    th()

            # ---- attention (v1 compute) ----
            def emit_scores(m, qv):
                """Transposed scores + exp for one query view.

                Returns [(kv, et)] where et is [128 keys, 1024] bf16 laid
                out (2h+j)*256 columns (h = head in pair, j = key chunk).
                """
                kT = qk_tiles[("k", m)]
                qT = qk_tiles[("q", m)]
                qs = slice(qv * 256, (qv + 1) * 256)
                ets = []
                for kv in _allowed(qv):
                    mark(f"scores{m}v{qv}kv{kv}")
                    pss = psum_e.tile([128, 1024], f32, tag="pse",
                                      name="pss")
                    for j in range(2):
                        kc = 2 * kv + j
                        for h in range(2):   # h inner: row groups alternate
                            nc.tensor.matmul(
                                pss[:, (2 * h + j) * 256:
                                    (2 * h + j + 1) * 256],
                                kT[64 * h:64 * (h + 1),
                                   kc * 128:(kc + 1) * 128],
                                qT[64 * h:64 * (h + 1), qs],
                                start=True, stop=True)
                    et = exp_pool.tile([128, 1024], bf16, tag="exp",
                                       name="et")
                    nc.scalar.activation(et[:], pss[:], EXP,
                                         scale=float(SCALE))
                    ets.append((kv, et))
                    fill()
                return ets

            def emit_pv(m, qv, ets):
                """Natural-layout PV + partition-parallel normalize."""
                kvs = _allowed(qv)
                rp = small_pool.tile([128, 4], f32, tag="rp", bufs=4,
                                     name="rp")
                for h in range(2):
                    hh = 2 * m + h
                    for qc in range(2):
                        mark(f"pv{m}v{qv}h{h}q{qc}")
                        g = 2 * h + qc
                        pg = psum_v.tile([128, 65], f32, tag="ppv",
                                         name=f"pg{g}")
                        for i, (kv, et) in enumerate(ets):
                            for j in range(2):
                                kc = 2 * kv + j
                                nc.tensor.matmul(
                                    pg[:],
                                    et[:, (2 * h + j) * 256 + qc * 128:
                                       (2 * h + j) * 256 + qc * 128 + 128],
                                    va[kc][:, hh * 65:(hh + 1) * 65],
                                    start=(i == 0 and j == 0),
                                    stop=(i == len(kvs) - 1 and j == 1))
                        sc = qv * 2 + qc
                        nc.vector.reciprocal(rp[:, g:g + 1], pg[:, 64:65])
                        nc.vector.tensor_scalar_mul(
                            on_tiles[sc][:, hh * 64:(hh + 1) * 64],
                            pg[:, 0:64],
                            rp[:, g:g + 1])

            def emit_trans_yproj(qv):
                """Transpose both m's attention out for views qv's two
                token chunks, then project + store them."""
                mark(f"typroj{qv}")
                ys = ysb_pool.tile([128, 2 * C], bf16, tag="ysb", name="ysb")
                for i in range(2):
                    sc = 2 * qv + i
                    for half in range(2):
                        pt = psum_j.tile([128, 128], bf16, tag="ppj",
                                         name="pt")
                        nc.tensor.transpose(
                            pt[:],
                            on_tiles[sc][:, half * 128:(half + 1) * 128],
                            ident[:])
                        nc.vector.tensor_copy(
                            ot_tiles[half][:, sc * 128:(sc + 1) * 128],
                            pt[:])
                    for n in range(2):
                        ps = psum_j.tile([128, 512], f32, tag="ppj",
                                         name="psy")
                        for k in range(2):
                            nc.tensor.matmul(
                                ps[:],
                                ot_tiles[k][:, sc * 128:(sc + 1) * 128],
                                wp_t[k][:, n * 512:(n + 1) * 512],
                                start=(k == 0), stop=(k == 1))
                        nc.vector.tensor_copy(
                            ys[:, i * C + n * 512:i * C + (n + 1) * 512],
                            ps[:])
                ycount[0] += 1
                # late stores alternate queues so the final two 512KB
                # transfers drain in parallel (scalar is done with exp)
                q = nc.scalar if (ycount[0] >= 5 and ycount[0] % 2 == 0) \
                    else nc.sync
                q.dma_start(
                    y[2 * qv * 128:(2 * qv + 2) * 128, :].rearrange(
                        "(a p) c -> p a c", p=128),
                    ys[:].rearrange("p (a c) -> p a c", c=C))

            ycount = [0]

            def emit_attn(m, qv):
                ets = emit_scores(m, qv)
                fill()
                emit_pv(m, qv, ets)

            # ---- fillers: v chunks + m=1 q/k projections ----
            for sc in range(0, 8):
                enqueue(("v", sc), lambda sc=sc: emit_v_chunk(sc))
            for qi in range(4):
                enqueue(("qk", qi), lambda nm="qk"[qi % 2], n=qi // 2:
                        emit_qk_proj_chunk(1, nm, n))
            for sc in range(8, 12):
                enqueue(("v", sc), lambda sc=sc: emit_v_chunk(sc))
            for qi in range(4, 8):
                enqueue(("qk", qi), lambda nm="qk"[qi % 2], n=qi // 2:
                        emit_qk_proj_chunk(1, nm, n))
            for sc in range(12, SC):
                enqueue(("v", sc), lambda sc=sc: emit_v_chunk(sc))

            def ensure_va(qv):
                for kv in _allowed(qv):
                    ensure(("v", 2 * kv))
                    ensure(("v", 2 * kv + 1))

            # ---- schedule ----
            # m=0: project q/k chunk n just-in-time, then the two views
            # whose queries live in chunk n. Scores for both views go
            # first (feeds the scalar engine early); their PVs follow so
            # a late va chunk can't stall the PE.
            for n in range(NS):
                emit_qk_proj_chunk(0, "q", n)
                emit_qk_proj_chunk(0, "k", n)
                for qv in (2 * n, 2 * n + 1):
                    ensure_va(qv)
                    emit_attn(0, qv)
                    fill()
            # m=1: big views first, tiny views (0,1) last for a short tail
            for qv in (2, 3, 4, 5, 6, 7, 1, 0):
                ensure(("qk", 2 * (qv // 2)))       # q chunk qv//2
                for n2 in range(qv // 2 + 1):
                    ensure(("qk", 2 * n2 + 1))      # k chunks <= qv//2
                ensure_va(qv)
                emit_attn(1, qv)
                fill()
                enqueue(("ty", qv), lambda qv=qv: emit_trans_yproj(qv))
            while order:
                fill()

    nc.compile()
    return nc


def _get_compiled():
    if "nc" not in _compiled:
        _compiled["nc"] = build()
    return _compiled["nc"]


def make_in_maps(x, Wq, Wk, Wv, Wp):
    xf = np.asarray(x, np.float32).reshape(B, S, C)
    in_maps = []
    for c in range(N_CORES):
        b, g = divmod(c, HPC)
        hs = slice(g * CPB, (g + 1) * CPB)
        bf = ml_dtypes.bfloat16
        in_maps.append({
            "xT": np.ascontiguousarray(xf[b].T).astype(bf),
            "wqT": np.ascontiguousarray(np.asarray(Wq, np.float32)[hs].T).astype(bf),
            "wkT": np.ascontiguousarray(np.asarray(Wk, np.float32)[hs].T).astype(bf),
            "wvT": np.ascontiguousarray(np.asarray(Wv, np.float32)[hs].T).astype(bf),
            "wpT": np.ascontiguousarray(np.asarray(Wp, np.float32)[:, hs].T).astype(bf),
        })
    return in_maps


def kernel(x, Wq, Wk, Wv, Wp, bp, _trace=False, _tmpdir=None):
    global LAST_RESULTS
    from concourse import bass_utils

    nc = _get_compiled()
    in_maps = make_in_maps(x, Wq, Wk, Wv, Wp)
    kwargs = {}
    if _trace:
        kwargs = {"trace": True, "tmpdir": _tmpdir}
    res = bass_utils.run_bass_kernel_spmd(
        nc, in_maps, core_ids=list(range(N_CORES)), **kwargs)
    LAST_RESULTS = res
    yout = np.zeros((B, S, C), np.float32)
    for c in range(N_CORES):
        yout[c // HPC] += res.results[c]["y"].astype(np.float32)
    yout += np.asarray(bp, np.float32).reshape(1, 1, C)
    return yout.reshape(B, V, L, C)
